# revision 13
# baseline (speedup 1.0000x reference)
"""Trainium2 Bass kernel for dynamic low-pass filter decomposition.

Module: global-avg-pool -> 1x1 conv -> BN -> softmax over 3x3 taps gives a
per-(sample, group) 3x3 kernel; applied as a reflect-padded depthwise conv
over x; returns (low, x - low).

Sharding: data-parallel over batch n=8 across 8 NeuronCores (1 sample/core).

Per-core layout: partition p = h*64 + c (h = row-half of the image, c =
channel).  Each partition holds 98 rows x 192 cols of its (channel, half)
with one halo row above/below (reflection resolved at DMA time by source row
choice) plus a 1-element front/back pad so tap-shifted views stay in bounds.

The 9-tap weighted sum runs on the TensorEngine as 9 diagonal fp32r matmuls
per 512-column chunk accumulating in PSUM; ScalarE copies low out of PSUM,
VectorE computes high = x - low and fixes the reflect columns at w=0/191.
The softmax "kernel generation" runs on-device from exact per-ST partial
sums (fp32), with BN folded into the 1x1 conv weights on the host.
"""
import sys
import os

sys.path.insert(0, "/opt/trn_rl_repo")

import numpy as np
from contextlib import ExitStack

import concourse.bass as bass
import concourse.tile as tile
from concourse import bacc, mybir
from concourse.bass_utils import run_bass_kernel_spmd

dt = mybir.dt
f32 = dt.float32

KS = 3
GROUP = 8
IC = 64
BN_EPS = 1e-5
N = 8
H = W = 192
RH = 96                 # rows per half-image
NB = 98 * W             # buffer elems per partition (98 rows of 192)
PAD = 1                 # front pad elems (also 1 at the back)
NST = 12                # input-phase tiles (small => reduces chase landing)
STW = 1536              # cols per input tile
CH = 512                # cols per chunk (one PSUM bank)
ST_ROWS = [16, 16, 16, 16, 16, 8, 8]   # compute super-tile heights (rows)


def _build_program():
    """Trace the SPMD Bass program (same for every core)."""
    nc = bacc.Bacc("TRN2", target_bir_lowering=False, debug=False,
                   num_devices=N)

    x_d = nc.dram_tensor("x", [64, H, W], dt.float32r, kind="ExternalInput")
    at_d = nc.dram_tensor("at128", [128, 72], f32, kind="ExternalInput")
    b_d = nc.dram_tensor("b72", [72, 1], f32, kind="ExternalInput")
    r9_d = nc.dram_tensor("r9", [72, 9], f32, kind="ExternalInput")
    g_d = nc.dram_tensor("g728", [72, 8], f32, kind="ExternalInput")
    h_d = nc.dram_tensor("h8128", [8, 128], f32, kind="ExternalInput")
    eye_d = nc.dram_tensor("eye", [128, 128], f32, kind="ExternalInput")
    low_d = nc.dram_tensor("low", [64, H, W], f32, kind="ExternalOutput")
    high_d = nc.dram_tensor("high", [64, H, W], f32, kind="ExternalOutput")

    xt_dram = x_d.ap()

    def dram_flat(tensor, base, inner):
        """Flat (128, inner) AP over DRAM: partition p = c*2 + h covers
        x.flat[p*18432 + base : ... + inner].  Flat leading-dim-128 APs get
        the full 16-engine DMA spray (~305 GB/s); (h,c)-interleaved ones
        only engage 2 engines (~53 GB/s measured)."""
        return bass.AP(tensor, base, [[RH * W, 128], [1, inner]])

    with tile.TileContext(nc) as tc, ExitStack() as ctx:
        cpool = ctx.enter_context(tc.tile_pool(name="consts", bufs=1))
        xpool = ctx.enter_context(tc.tile_pool(name="x", bufs=1))
        wpool = ctx.enter_context(tc.tile_pool(name="w", bufs=1))
        spool = ctx.enter_context(tc.tile_pool(name="stage", bufs=3))

        # ---- x ST loads FIRST (queue FIFO position = landing time);
        # consts/halos after, so they don't delay the reduces ----
        xt = xpool.tile([128, PAD + NB + 1], dt.float32r)
        partials_v = wpool.tile([128, NST // 2], f32)
        partials_a = wpool.tile([128, NST // 2], f32)
        rscratch = wpool.tile([128, STW], f32)
        for s in range(NST):
            a = PAD + W + STW * s
            eng = nc.sync if s % 2 == 0 else nc.scalar
            eng.dma_start(xt[:, a:a + STW],
                          dram_flat(xt_dram.tensor, STW * s, STW))
        for s in range(NST):
            a = PAD + W + STW * s
            if s % 2 == 0:
                nc.vector.tensor_reduce(partials_v[:, s // 2:s // 2 + 1],
                                        xt[:, a:a + STW].bitcast(f32),
                                        axis=mybir.AxisListType.X,
                                        op=mybir.AluOpType.add)
            else:
                nc.scalar.activation(rscratch[:],
                                     xt[:, a:a + STW].bitcast(f32),
                                     mybir.ActivationFunctionType.Copy,
                                     accum_out=partials_a[:, s // 2:s // 2 + 1])

        # ---- constant + halo loads (needed from the weight chain on) ----
        at_s = cpool.tile([128, 72], f32)
        b_s = cpool.tile([72, 1], f32)
        r9_s = cpool.tile([72, 9], f32)
        g_s = cpool.tile([72, 8], f32)
        h_s = cpool.tile([8, 128], f32)
        eye_s = cpool.tile([128, 128], f32)
        for t, d in ((at_s, at_d), (b_s, b_d), (r9_s, r9_d), (g_s, g_d),
                     (h_s, h_d), (eye_s, eye_d)):
            nc.scalar.dma_start(t[:], d.ap())
        # halo row 0 <- image rows {1 (reflect), 95}[h]
        nc.sync.dma_start(xt[:, PAD:PAD + W],
                          bass.AP(xt_dram.tensor, W,
                                  [[H * W, 64], [94 * W, 2], [1, W]]))
        # halo row 97 <- image rows {96, 190 (reflect)}[h]
        nc.sync.dma_start(xt[:, PAD + 97 * W:PAD + 98 * W],
                          bass.AP(xt_dram.tensor, 96 * W,
                                  [[H * W, 64], [94 * W, 2], [1, W]]))

        # ---- weight generation ----
        sum_v = wpool.tile([128, 1], f32)
        nc.vector.tensor_reduce(sum_v[:], partials_v[:],
                                axis=mybir.AxisListType.X,
                                op=mybir.AluOpType.add)
        sum_a = wpool.tile([128, 1], f32)
        nc.vector.tensor_reduce(sum_a[:], partials_a[:],
                                axis=mybir.AxisListType.X,
                                op=mybir.AluOpType.add)
        sum128 = wpool.tile([128, 1], f32)
        nc.vector.tensor_add(sum128[:], sum_v[:], sum_a[:])
        with tc.tile_pool(name="wpsum", bufs=1,
                          space=bass.MemorySpace.PSUM) as wpsum:
            lf_p = wpsum.tile([72, 1], f32, tag="lf")
            nc.tensor.matmul(lf_p[:], at_s[:], sum128[:])
            e72 = wpool.tile([72, 1], f32)
            nc.scalar.activation(e72[:], lf_p[:],
                                 mybir.ActivationFunctionType.Exp,
                                 bias=b_s[:, 0:1], scale=1.0)
            rhsw = wpool.tile([72, 9], f32)
            nc.vector.tensor_scalar_mul(rhsw[:], r9_s[:], e72[:, 0:1])
            w89_p = wpsum.tile([8, 9], f32, tag="w89")
            nc.tensor.matmul(w89_p[:], g_s[:], rhsw[:])
            s8 = wpool.tile([8, 1], f32)
            nc.vector.tensor_reduce(s8[:], w89_p[:],
                                    axis=mybir.AxisListType.X,
                                    op=mybir.AluOpType.add)
            r8 = wpool.tile([8, 1], f32)
            nc.vector.reciprocal(r8[:], s8[:])
            w89s = wpool.tile([8, 9], f32)
            nc.vector.tensor_scalar_mul(w89s[:], w89_p[:], r8[:, 0:1])
            wbig_p = wpsum.tile([128, 9], f32, tag="wbig")
            nc.tensor.matmul(wbig_p[:], h_s[:], w89s[:])
            w128 = wpool.tile([128, 9], f32)
            nc.scalar.copy(w128[:], wbig_p[:])

        # diagonal weight matrices, one tile per tap (separate tiles so
        # the first matmul only waits for its own diagonal); scalars read
        # straight from PSUM so PE needn't wait for the w128 SBUF copy
        diag = [wpool.tile([128, 128], dt.float32r, name=f"diag{k}")
                for k in range(9)]
        for k in range(9):
            nc.vector.tensor_scalar_mul(diag[k][:], eye_s[:],
                                        wbig_p[:, k:k + 1])

        # ---- main loop ----
        with tc.tile_pool(name="psum", bufs=8,
                          space=bass.MemorySpace.PSUM) as psum:
            r0 = 0
            for s, rows in enumerate(ST_ROWS):
                stw = rows * W
                nch = stw // CH
                base = PAD + W + r0 * W
                acc = [psum.tile([128, CH], f32, tag="acc", name=f"acc{s}_{i}")
                       for i in range(nch)]
                taps = range(9) if s % 2 == 0 else range(8, -1, -1)
                taps = list(taps)
                for k in taps:
                    di, dj = k // 3, k % 3
                    shift = (di - 1) * W + (dj - 1)
                    for ch in range(nch):
                        off = base + CH * ch + shift
                        nc.tensor.matmul(acc[ch][:], diag[k][:],
                                         xt[:, off:off + CH],
                                         start=(k == taps[0]),
                                         stop=(k == taps[-1]))
                low_st = spool.tile([128, stw], f32, tag="low",
                                    padded_shape=[128, 3072])
                for ch in range(nch):
                    dst = low_st[:, CH * ch:CH * (ch + 1)]
                    if ch % 2 == 0:
                        nc.scalar.copy(dst, acc[ch][:])
                    else:
                        nc.vector.tensor_copy(dst, acc[ch][:])
                # edge-column fixes (reflect at w=0 and w=191), both columns
                # per op via a stride-(wr-wl) length-2 inner dim
                out_ap = low_st[:, 0:stw].rearrange(
                    "p (r w) -> p r w", w=W)[:, :, 0:W:W - 1]
                for k in range(9):
                    di, dj = k // 3, k % 3
                    wl = (1, 0, 1)[dj]
                    wr = (190, 191, 190)[dj]
                    vb = PAD + (r0 + di) * W + wl
                    view = xt[:, vb:vb + rows * W].bitcast(f32).rearrange(
                        "p (r w) -> p r w", w=W)[:, :, 0:wr - wl + 1:wr - wl]
                    if k == 0:
                        nc.vector.tensor_scalar_mul(out_ap, view,
                                                    w128[:, 0:1])
                    else:
                        nc.vector.scalar_tensor_tensor(
                            out_ap, view, w128[:, k:k + 1], out_ap,
                            op0=mybir.AluOpType.mult,
                            op1=mybir.AluOpType.add)
                high_st = spool.tile([128, stw], f32, tag="high",
                                     padded_shape=[128, 3072])
                nc.vector.tensor_tensor(high_st[:],
                                        xt[:, base:base + stw].bitcast(f32),
                                        low_st[:],
                                        op=mybir.AluOpType.subtract)
                nc.scalar.dma_start(
                    dram_flat(low_d.ap().tensor, r0 * W, stw), low_st[:])
                nc.sync.dma_start(
                    dram_flat(high_d.ap().tensor, r0 * W, stw), high_st[:])
                r0 += rows

    nc.compile()
    return nc


def _enable_ldw_opt():
    """walrus emits one LDWEIGHTS per matmul with --enable-ldw-opt=false
    (72us of PE time for our 330 matmuls, mostly redundant reloads of the
    same diagonal).  Rewrite the flag on the compiler command line."""
    import concourse.bass_utils as BU
    if getattr(BU, "_ldw_patched", False):
        return
    orig = BU.run_command

    def patched(cmd, *a, **kw):
        cmd = [c.replace("--enable-ldw-opt=false", "--enable-ldw-opt=true")
               if isinstance(c, str) else c for c in cmd]
        return orig(cmd, *a, **kw)

    BU.run_command = patched
    BU._ldw_patched = True
    # bir_verify_and_optimise captured run_command at def time? (no - it
    # resolves the module global at call time, so the wrap is enough)


_nc_cache = None


def _get_program():
    global _nc_cache
    if _nc_cache is None:
        _enable_ldw_opt()
        _nc_cache = _build_program()
    return _nc_cache


def _host_consts(conv_w, bn_gamma, bn_beta, bn_mean, bn_var):
    s_a = bn_gamma / np.sqrt(bn_var + BN_EPS)
    b72 = (bn_beta - bn_mean * s_a).astype(np.float32).reshape(72, 1)
    A = (conv_w * s_a[:, None]) / np.float32(H * W)
    p = np.arange(128)
    at128 = np.ascontiguousarray(A.T[p // 2]).astype(np.float32)  # (128, 72)
    oc = np.arange(72)
    r9 = (oc[:, None] % 9 == np.arange(9)[None, :]).astype(np.float32)
    g728 = (oc[:, None] // 9 == np.arange(8)[None, :]).astype(np.float32)
    h8128 = (np.arange(8)[:, None] == (p[None, :] // 16)).astype(np.float32)
    eye = np.eye(128, dtype=np.float32)
    return dict(at128=at128, b72=b72, r9=r9, g728=g728, h8128=h8128, eye=eye)


def kernel(x, conv_w, bn_gamma, bn_beta, bn_mean, bn_var):
    x = np.ascontiguousarray(np.asarray(x, dtype=np.float32))
    consts = _host_consts(np.asarray(conv_w, np.float32),
                          np.asarray(bn_gamma, np.float32),
                          np.asarray(bn_beta, np.float32),
                          np.asarray(bn_mean, np.float32),
                          np.asarray(bn_var, np.float32))
    nc = _get_program()
    in_maps = [dict(x=x[i], **consts) for i in range(N)]
    res = run_bass_kernel_spmd(nc, in_maps, list(range(N))).results
    low = np.stack([res[i]["low"] for i in range(N)])
    high = np.stack([res[i]["high"] for i in range(N)])
    return low, high


if __name__ == "__main__":
    rng = np.random.default_rng(0)
    demo = dict(
        x=rng.standard_normal((N, IC, H, W), dtype=np.float32),
        conv_w=rng.standard_normal((72, 64)).astype(np.float32),
        bn_gamma=np.ones(72, np.float32),
        bn_beta=np.zeros(72, np.float32),
        bn_mean=rng.standard_normal(72).astype(np.float32) * 0.1,
        bn_var=rng.uniform(0.5, 1.5, 72).astype(np.float32),
    )
    low, high = kernel(**demo)
    print("ok", low.shape, high.shape)


# revision 15
# speedup vs baseline: 1.0342x; 1.0342x over previous
"""Trainium2 Bass kernel for dynamic low-pass filter decomposition.

Module: global-avg-pool -> 1x1 conv -> BN -> softmax over 3x3 taps gives a
per-(sample, group) 3x3 kernel; applied as a reflect-padded depthwise conv
over x; returns (low, x - low).

Sharding: data-parallel over batch n=8 across 8 NeuronCores (1 sample/core).

Per-core layout: partition p = h*64 + c (h = row-half of the image, c =
channel).  Each partition holds 98 rows x 192 cols of its (channel, half)
with one halo row above/below (reflection resolved at DMA time by source row
choice) plus a 1-element front/back pad so tap-shifted views stay in bounds.

The 9-tap weighted sum runs on the TensorEngine as 9 diagonal fp32r matmuls
per 512-column chunk accumulating in PSUM; ScalarE copies low out of PSUM,
VectorE computes high = x - low and fixes the reflect columns at w=0/191.
The softmax "kernel generation" runs on-device from exact per-ST partial
sums (fp32), with BN folded into the 1x1 conv weights on the host.
"""
import sys
import os

sys.path.insert(0, "/opt/trn_rl_repo")

import numpy as np
from contextlib import ExitStack

import concourse.bass as bass
import concourse.tile as tile
from concourse import bacc, mybir
from concourse.bass_utils import run_bass_kernel_spmd

dt = mybir.dt
f32 = dt.float32

KS = 3
GROUP = 8
IC = 64
BN_EPS = 1e-5
N = 8
H = W = 192
RH = 96                 # rows per half-image
NB = 98 * W             # buffer elems per partition (98 rows of 192)
PAD = 1                 # front pad elems (also 1 at the back)
NST = 6                 # input-phase tiles
STW = 3072              # cols per input tile
CH = 512                # cols per chunk (one PSUM bank)
ST_ROWS = [16, 16, 16, 16, 16, 8, 8]   # compute super-tile heights (rows)


def _build_program():
    """Trace the SPMD Bass program (same for every core)."""
    nc = bacc.Bacc("TRN2", target_bir_lowering=False, debug=False,
                   num_devices=N)

    x_d = nc.dram_tensor("x", [64, H, W], dt.float32r, kind="ExternalInput")
    at_d = nc.dram_tensor("at128", [128, 72], f32, kind="ExternalInput")
    b_d = nc.dram_tensor("b72", [72, 1], f32, kind="ExternalInput")
    r9_d = nc.dram_tensor("r9", [72, 9], f32, kind="ExternalInput")
    g_d = nc.dram_tensor("g728", [72, 8], f32, kind="ExternalInput")
    h_d = nc.dram_tensor("h8128", [8, 128], f32, kind="ExternalInput")
    eye_d = nc.dram_tensor("eye", [128, 128], f32, kind="ExternalInput")
    low_d = nc.dram_tensor("low", [64, H, W], f32, kind="ExternalOutput")
    high_d = nc.dram_tensor("high", [64, H, W], f32, kind="ExternalOutput")

    xt_dram = x_d.ap()

    def dram_flat(tensor, base, inner):
        """Flat (128, inner) AP over DRAM: partition p = c*2 + h covers
        x.flat[p*18432 + base : ... + inner].  Flat leading-dim-128 APs get
        the full 16-engine DMA spray (~305 GB/s); (h,c)-interleaved ones
        only engage 2 engines (~53 GB/s measured)."""
        return bass.AP(tensor, base, [[RH * W, 128], [1, inner]])

    with tile.TileContext(nc) as tc, ExitStack() as ctx:
        cpool = ctx.enter_context(tc.tile_pool(name="consts", bufs=1))
        xpool = ctx.enter_context(tc.tile_pool(name="x", bufs=1))
        wpool = ctx.enter_context(tc.tile_pool(name="w", bufs=1))
        spool = ctx.enter_context(tc.tile_pool(name="stage", bufs=3))

        # ---- x ST loads FIRST (queue FIFO position = landing time);
        # consts/halos after, so they don't delay the reduces ----
        xt = xpool.tile([128, PAD + NB + 1], dt.float32r)
        partials_v = wpool.tile([128, NST // 2], f32)
        partials_a = wpool.tile([128, NST // 2], f32)
        rscratch = wpool.tile([128, STW], f32)
        for s in range(NST):
            a = PAD + W + STW * s
            eng = nc.sync if s < 3 else nc.scalar
            eng.dma_start(xt[:, a:a + STW],
                          dram_flat(xt_dram.tensor, STW * s, STW))
        # PE warm-up stream: keeps the tensor engine's p-state ramped while
        # the input streams in, so the real matmuls start at full clock
        warm = wpool.tile([128, 512], f32)
        nc.gpsimd.memset(warm[:], 0.0)
        with tc.tile_pool(name="warmp", bufs=1,
                          space=bass.MemorySpace.PSUM) as warmp:
            wp = warmp.tile([128, 512], f32)
            wr = warm[:].bitcast(dt.float32r)
            for i in range(140):
                nc.tensor.matmul(wp[:], wr[:, 0:128], wr, start=(i == 0),
                                 stop=(i == 139))
        for s in range(NST):
            a = PAD + W + STW * s
            if s < 3:
                nc.vector.tensor_reduce(partials_v[:, s:s + 1],
                                        xt[:, a:a + STW].bitcast(f32),
                                        axis=mybir.AxisListType.X,
                                        op=mybir.AluOpType.add)
            else:
                nc.scalar.activation(rscratch[:],
                                     xt[:, a:a + STW].bitcast(f32),
                                     mybir.ActivationFunctionType.Copy,
                                     accum_out=partials_a[:, s - 3:s - 2])

        # ---- constant + halo loads (needed from the weight chain on) ----
        at_s = cpool.tile([128, 72], f32)
        b_s = cpool.tile([72, 1], f32)
        r9_s = cpool.tile([72, 9], f32)
        g_s = cpool.tile([72, 8], f32)
        h_s = cpool.tile([8, 128], f32)
        eye_s = cpool.tile([128, 128], f32)
        for t, d in ((at_s, at_d), (b_s, b_d), (r9_s, r9_d), (g_s, g_d),
                     (h_s, h_d), (eye_s, eye_d)):
            nc.scalar.dma_start(t[:], d.ap())
        # halo row 0 <- image rows {1 (reflect), 95}[h]
        nc.sync.dma_start(xt[:, PAD:PAD + W],
                          bass.AP(xt_dram.tensor, W,
                                  [[H * W, 64], [94 * W, 2], [1, W]]))
        # halo row 97 <- image rows {96, 190 (reflect)}[h]
        nc.sync.dma_start(xt[:, PAD + 97 * W:PAD + 98 * W],
                          bass.AP(xt_dram.tensor, 96 * W,
                                  [[H * W, 64], [94 * W, 2], [1, W]]))

        # ---- weight generation ----
        sum_v = wpool.tile([128, 1], f32)
        nc.vector.tensor_reduce(sum_v[:], partials_v[:],
                                axis=mybir.AxisListType.X,
                                op=mybir.AluOpType.add)
        sum_a = wpool.tile([128, 1], f32)
        nc.vector.tensor_reduce(sum_a[:], partials_a[:],
                                axis=mybir.AxisListType.X,
                                op=mybir.AluOpType.add)
        sum128 = wpool.tile([128, 1], f32)
        nc.vector.tensor_add(sum128[:], sum_v[:], sum_a[:])
        with tc.tile_pool(name="wpsum", bufs=1,
                          space=bass.MemorySpace.PSUM) as wpsum:
            lf_p = wpsum.tile([72, 1], f32, tag="lf")
            nc.tensor.matmul(lf_p[:], at_s[:], sum128[:])
            e72 = wpool.tile([72, 1], f32)
            nc.scalar.activation(e72[:], lf_p[:],
                                 mybir.ActivationFunctionType.Exp,
                                 bias=b_s[:, 0:1], scale=1.0)
            rhsw = wpool.tile([72, 9], f32)
            nc.vector.tensor_scalar_mul(rhsw[:], r9_s[:], e72[:, 0:1])
            w89_p = wpsum.tile([8, 9], f32, tag="w89")
            nc.tensor.matmul(w89_p[:], g_s[:], rhsw[:])
            s8 = wpool.tile([8, 1], f32)
            nc.vector.tensor_reduce(s8[:], w89_p[:],
                                    axis=mybir.AxisListType.X,
                                    op=mybir.AluOpType.add)
            r8 = wpool.tile([8, 1], f32)
            nc.vector.reciprocal(r8[:], s8[:])
            w89s = wpool.tile([8, 9], f32)
            nc.vector.tensor_scalar_mul(w89s[:], w89_p[:], r8[:, 0:1])
            wbig_p = wpsum.tile([128, 9], f32, tag="wbig")
            nc.tensor.matmul(wbig_p[:], h_s[:], w89s[:])
            w128 = wpool.tile([128, 9], f32)
            nc.scalar.copy(w128[:], wbig_p[:])

        # diagonal weight matrices, one tile per tap (separate tiles so
        # the first matmul only waits for its own diagonal); scalars read
        # straight from PSUM so PE needn't wait for the w128 SBUF copy
        diag = [wpool.tile([128, 128], dt.float32r, name=f"diag{k}")
                for k in range(9)]
        for k in range(9):
            nc.vector.tensor_scalar_mul(diag[k][:], eye_s[:],
                                        wbig_p[:, k:k + 1])

        # ---- main loop ----
        with tc.tile_pool(name="psum", bufs=8,
                          space=bass.MemorySpace.PSUM) as psum:
            r0 = 0
            for s, rows in enumerate(ST_ROWS):
                stw = rows * W
                nch = stw // CH
                base = PAD + W + r0 * W
                acc = [psum.tile([128, CH], f32, tag="acc", name=f"acc{s}_{i}")
                       for i in range(nch)]
                taps = range(9) if s % 2 == 0 else range(8, -1, -1)
                taps = list(taps)
                for k in taps:
                    di, dj = k // 3, k % 3
                    shift = (di - 1) * W + (dj - 1)
                    for ch in range(nch):
                        off = base + CH * ch + shift
                        nc.tensor.matmul(acc[ch][:], diag[k][:],
                                         xt[:, off:off + CH],
                                         start=(k == taps[0]),
                                         stop=(k == taps[-1]))
                low_st = spool.tile([128, stw], f32, tag="low",
                                    padded_shape=[128, 3072])
                for ch in range(nch):
                    dst = low_st[:, CH * ch:CH * (ch + 1)]
                    if ch % 2 == 0:
                        nc.scalar.copy(dst, acc[ch][:])
                    else:
                        nc.vector.tensor_copy(dst, acc[ch][:])
                # edge-column fixes (reflect at w=0 and w=191), both columns
                # per op via a stride-(wr-wl) length-2 inner dim
                out_ap = low_st[:, 0:stw].rearrange(
                    "p (r w) -> p r w", w=W)[:, :, 0:W:W - 1]
                for k in range(9):
                    di, dj = k // 3, k % 3
                    wl = (1, 0, 1)[dj]
                    wr = (190, 191, 190)[dj]
                    vb = PAD + (r0 + di) * W + wl
                    view = xt[:, vb:vb + rows * W].bitcast(f32).rearrange(
                        "p (r w) -> p r w", w=W)[:, :, 0:wr - wl + 1:wr - wl]
                    if k == 0:
                        nc.vector.tensor_scalar_mul(out_ap, view,
                                                    w128[:, 0:1])
                    else:
                        nc.vector.scalar_tensor_tensor(
                            out_ap, view, w128[:, k:k + 1], out_ap,
                            op0=mybir.AluOpType.mult,
                            op1=mybir.AluOpType.add)
                high_st = spool.tile([128, stw], f32, tag="high",
                                     padded_shape=[128, 3072])
                nc.vector.tensor_tensor(high_st[:],
                                        xt[:, base:base + stw].bitcast(f32),
                                        low_st[:],
                                        op=mybir.AluOpType.subtract)
                nc.scalar.dma_start(
                    dram_flat(low_d.ap().tensor, r0 * W, stw), low_st[:])
                nc.sync.dma_start(
                    dram_flat(high_d.ap().tensor, r0 * W, stw), high_st[:])
                r0 += rows

    nc.compile()
    return nc


def _enable_ldw_opt():
    """walrus emits one LDWEIGHTS per matmul with --enable-ldw-opt=false
    (72us of PE time for our 330 matmuls, mostly redundant reloads of the
    same diagonal).  Rewrite the flag on the compiler command line."""
    import concourse.bass_utils as BU
    if getattr(BU, "_ldw_patched", False):
        return
    orig = BU.run_command

    def patched(cmd, *a, **kw):
        cmd = [c.replace("--enable-ldw-opt=false", "--enable-ldw-opt=true")
               if isinstance(c, str) else c for c in cmd]
        return orig(cmd, *a, **kw)

    BU.run_command = patched
    BU._ldw_patched = True
    # bir_verify_and_optimise captured run_command at def time? (no - it
    # resolves the module global at call time, so the wrap is enough)


_nc_cache = None


def _get_program():
    global _nc_cache
    if _nc_cache is None:
        _enable_ldw_opt()
        _nc_cache = _build_program()
    return _nc_cache


def _host_consts(conv_w, bn_gamma, bn_beta, bn_mean, bn_var):
    s_a = bn_gamma / np.sqrt(bn_var + BN_EPS)
    b72 = (bn_beta - bn_mean * s_a).astype(np.float32).reshape(72, 1)
    A = (conv_w * s_a[:, None]) / np.float32(H * W)
    p = np.arange(128)
    at128 = np.ascontiguousarray(A.T[p // 2]).astype(np.float32)  # (128, 72)
    oc = np.arange(72)
    r9 = (oc[:, None] % 9 == np.arange(9)[None, :]).astype(np.float32)
    g728 = (oc[:, None] // 9 == np.arange(8)[None, :]).astype(np.float32)
    h8128 = (np.arange(8)[:, None] == (p[None, :] // 16)).astype(np.float32)
    eye = np.eye(128, dtype=np.float32)
    return dict(at128=at128, b72=b72, r9=r9, g728=g728, h8128=h8128, eye=eye)


def kernel(x, conv_w, bn_gamma, bn_beta, bn_mean, bn_var):
    x = np.ascontiguousarray(np.asarray(x, dtype=np.float32))
    consts = _host_consts(np.asarray(conv_w, np.float32),
                          np.asarray(bn_gamma, np.float32),
                          np.asarray(bn_beta, np.float32),
                          np.asarray(bn_mean, np.float32),
                          np.asarray(bn_var, np.float32))
    nc = _get_program()
    in_maps = [dict(x=x[i], **consts) for i in range(N)]
    res = run_bass_kernel_spmd(nc, in_maps, list(range(N))).results
    low = np.stack([res[i]["low"] for i in range(N)])
    high = np.stack([res[i]["high"] for i in range(N)])
    return low, high


if __name__ == "__main__":
    rng = np.random.default_rng(0)
    demo = dict(
        x=rng.standard_normal((N, IC, H, W), dtype=np.float32),
        conv_w=rng.standard_normal((72, 64)).astype(np.float32),
        bn_gamma=np.ones(72, np.float32),
        bn_beta=np.zeros(72, np.float32),
        bn_mean=rng.standard_normal(72).astype(np.float32) * 0.1,
        bn_var=rng.uniform(0.5, 1.5, 72).astype(np.float32),
    )
    low, high = kernel(**demo)
    print("ok", low.shape, high.shape)


# revision 16
# speedup vs baseline: 1.0349x; 1.0008x over previous
"""Trainium2 Bass kernel for dynamic low-pass filter decomposition.

Module: global-avg-pool -> 1x1 conv -> BN -> softmax over 3x3 taps gives a
per-(sample, group) 3x3 kernel; applied as a reflect-padded depthwise conv
over x; returns (low, x - low).

Sharding: data-parallel over batch n=8 across 8 NeuronCores (1 sample/core).

Per-core layout: partition p = h*64 + c (h = row-half of the image, c =
channel).  Each partition holds 98 rows x 192 cols of its (channel, half)
with one halo row above/below (reflection resolved at DMA time by source row
choice) plus a 1-element front/back pad so tap-shifted views stay in bounds.

The 9-tap weighted sum runs on the TensorEngine as 9 diagonal fp32r matmuls
per 512-column chunk accumulating in PSUM; ScalarE copies low out of PSUM,
VectorE computes high = x - low and fixes the reflect columns at w=0/191.
The softmax "kernel generation" runs on-device from exact per-ST partial
sums (fp32), with BN folded into the 1x1 conv weights on the host.
"""
import sys
import os

sys.path.insert(0, "/opt/trn_rl_repo")

import numpy as np
from contextlib import ExitStack

import concourse.bass as bass
import concourse.tile as tile
from concourse import bacc, mybir
from concourse.bass_utils import run_bass_kernel_spmd

dt = mybir.dt
f32 = dt.float32

KS = 3
GROUP = 8
IC = 64
BN_EPS = 1e-5
N = 8
H = W = 192
RH = 96                 # rows per half-image
NB = 98 * W             # buffer elems per partition (98 rows of 192)
PAD = 1                 # front pad elems (also 1 at the back)
NST = 6                 # input-phase tiles
STW = 3072              # cols per input tile
CH = 512                # cols per chunk (one PSUM bank)
ST_ROWS = [16, 16, 16, 16, 16, 8, 8]   # compute super-tile heights (rows)


def _build_program():
    """Trace the SPMD Bass program (same for every core)."""
    nc = bacc.Bacc("TRN2", target_bir_lowering=False, debug=False,
                   num_devices=N)

    x_d = nc.dram_tensor("x", [64, H, W], dt.float32r, kind="ExternalInput")
    at_d = nc.dram_tensor("at128", [128, 72], f32, kind="ExternalInput")
    b_d = nc.dram_tensor("b72", [72, 1], f32, kind="ExternalInput")
    r9_d = nc.dram_tensor("r9", [72, 9], f32, kind="ExternalInput")
    g_d = nc.dram_tensor("g728", [72, 8], f32, kind="ExternalInput")
    h_d = nc.dram_tensor("h8128", [8, 128], f32, kind="ExternalInput")
    eye_d = nc.dram_tensor("eye", [128, 128], f32, kind="ExternalInput")
    low_d = nc.dram_tensor("low", [64, H, W], f32, kind="ExternalOutput")
    high_d = nc.dram_tensor("high", [64, H, W], f32, kind="ExternalOutput")

    xt_dram = x_d.ap()

    def dram_flat(tensor, base, inner):
        """Flat (128, inner) AP over DRAM: partition p = c*2 + h covers
        x.flat[p*18432 + base : ... + inner].  Flat leading-dim-128 APs get
        the full 16-engine DMA spray (~305 GB/s); (h,c)-interleaved ones
        only engage 2 engines (~53 GB/s measured)."""
        return bass.AP(tensor, base, [[RH * W, 128], [1, inner]])

    with tile.TileContext(nc) as tc, ExitStack() as ctx:
        cpool = ctx.enter_context(tc.tile_pool(name="consts", bufs=1))
        xpool = ctx.enter_context(tc.tile_pool(name="x", bufs=1))
        wpool = ctx.enter_context(tc.tile_pool(name="w", bufs=1))
        spool = ctx.enter_context(tc.tile_pool(name="stage", bufs=3))

        # ---- x ST loads FIRST (queue FIFO position = landing time);
        # consts/halos after, so they don't delay the reduces ----
        xt = xpool.tile([128, PAD + NB + 1], dt.float32r)
        partials_v = wpool.tile([128, NST // 2], f32)
        partials_a = wpool.tile([128, NST // 2], f32)
        rscratch = wpool.tile([128, STW], f32)
        for s in range(NST):
            a = PAD + W + STW * s
            eng = nc.sync if s < 3 else nc.scalar
            eng.dma_start(xt[:, a:a + STW],
                          dram_flat(xt_dram.tensor, STW * s, STW))
        # PE warm-up stream: keeps the tensor engine's p-state ramped while
        # the input streams in, so the real matmuls start at full clock
        warm = wpool.tile([128, 512], f32)
        nc.gpsimd.memset(warm[:], 0.0)
        with tc.tile_pool(name="warmp", bufs=1,
                          space=bass.MemorySpace.PSUM) as warmp:
            wp = warmp.tile([128, 512], f32)
            wr = warm[:].bitcast(dt.float32r)
            for i in range(60):
                nc.tensor.matmul(wp[:], wr[:, 0:128], wr, start=(i == 0),
                                 stop=(i == 59))
        for s in range(NST):
            a = PAD + W + STW * s
            if s < 3:
                nc.vector.tensor_reduce(partials_v[:, s:s + 1],
                                        xt[:, a:a + STW].bitcast(f32),
                                        axis=mybir.AxisListType.X,
                                        op=mybir.AluOpType.add)
            else:
                nc.scalar.activation(rscratch[:],
                                     xt[:, a:a + STW].bitcast(f32),
                                     mybir.ActivationFunctionType.Copy,
                                     accum_out=partials_a[:, s - 3:s - 2])

        # ---- constant + halo loads (needed from the weight chain on) ----
        at_s = cpool.tile([128, 72], f32)
        b_s = cpool.tile([72, 1], f32)
        r9_s = cpool.tile([72, 9], f32)
        g_s = cpool.tile([72, 8], f32)
        h_s = cpool.tile([8, 128], f32)
        eye_s = cpool.tile([128, 128], f32)
        for t, d in ((at_s, at_d), (b_s, b_d), (r9_s, r9_d), (g_s, g_d),
                     (h_s, h_d), (eye_s, eye_d)):
            nc.scalar.dma_start(t[:], d.ap())
        # halo row 0 <- image rows {1 (reflect), 95}[h]
        nc.sync.dma_start(xt[:, PAD:PAD + W],
                          bass.AP(xt_dram.tensor, W,
                                  [[H * W, 64], [94 * W, 2], [1, W]]))
        # halo row 97 <- image rows {96, 190 (reflect)}[h]
        nc.sync.dma_start(xt[:, PAD + 97 * W:PAD + 98 * W],
                          bass.AP(xt_dram.tensor, 96 * W,
                                  [[H * W, 64], [94 * W, 2], [1, W]]))

        # ---- weight generation ----
        sum_v = wpool.tile([128, 1], f32)
        nc.vector.tensor_reduce(sum_v[:], partials_v[:],
                                axis=mybir.AxisListType.X,
                                op=mybir.AluOpType.add)
        sum_a = wpool.tile([128, 1], f32)
        nc.vector.tensor_reduce(sum_a[:], partials_a[:],
                                axis=mybir.AxisListType.X,
                                op=mybir.AluOpType.add)
        sum128 = wpool.tile([128, 1], f32)
        nc.vector.tensor_add(sum128[:], sum_v[:], sum_a[:])
        with tc.tile_pool(name="wpsum", bufs=1,
                          space=bass.MemorySpace.PSUM) as wpsum:
            lf_p = wpsum.tile([72, 1], f32, tag="lf")
            nc.tensor.matmul(lf_p[:], at_s[:], sum128[:])
            e72 = wpool.tile([72, 1], f32)
            nc.scalar.activation(e72[:], lf_p[:],
                                 mybir.ActivationFunctionType.Exp,
                                 bias=b_s[:, 0:1], scale=1.0)
            rhsw = wpool.tile([72, 9], f32)
            nc.vector.tensor_scalar_mul(rhsw[:], r9_s[:], e72[:, 0:1])
            w89_p = wpsum.tile([8, 9], f32, tag="w89")
            nc.tensor.matmul(w89_p[:], g_s[:], rhsw[:])
            s8 = wpool.tile([8, 1], f32)
            nc.vector.tensor_reduce(s8[:], w89_p[:],
                                    axis=mybir.AxisListType.X,
                                    op=mybir.AluOpType.add)
            r8 = wpool.tile([8, 1], f32)
            nc.vector.reciprocal(r8[:], s8[:])
            w89s = wpool.tile([8, 9], f32)
            nc.vector.tensor_scalar_mul(w89s[:], w89_p[:], r8[:, 0:1])
            wbig_p = wpsum.tile([128, 9], f32, tag="wbig")
            nc.tensor.matmul(wbig_p[:], h_s[:], w89s[:])
            w128 = wpool.tile([128, 9], f32)
            nc.scalar.copy(w128[:], wbig_p[:])

        # diagonal weight matrices, one tile per tap (separate tiles so
        # the first matmul only waits for its own diagonal); scalars read
        # straight from PSUM so PE needn't wait for the w128 SBUF copy
        diag = [wpool.tile([128, 128], dt.float32r, name=f"diag{k}")
                for k in range(9)]
        for k in range(9):
            nc.vector.tensor_scalar_mul(diag[k][:], eye_s[:],
                                        wbig_p[:, k:k + 1])

        # ---- main loop ----
        with tc.tile_pool(name="psum", bufs=8,
                          space=bass.MemorySpace.PSUM) as psum:
            r0 = 0
            for s, rows in enumerate(ST_ROWS):
                stw = rows * W
                nch = stw // CH
                base = PAD + W + r0 * W
                acc = [psum.tile([128, CH], f32, tag="acc", name=f"acc{s}_{i}")
                       for i in range(nch)]
                taps = range(9) if s % 2 == 0 else range(8, -1, -1)
                taps = list(taps)
                for k in taps:
                    di, dj = k // 3, k % 3
                    shift = (di - 1) * W + (dj - 1)
                    for ch in range(nch):
                        off = base + CH * ch + shift
                        nc.tensor.matmul(acc[ch][:], diag[k][:],
                                         xt[:, off:off + CH],
                                         start=(k == taps[0]),
                                         stop=(k == taps[-1]))
                low_st = spool.tile([128, stw], f32, tag="low",
                                    padded_shape=[128, 3072])
                for ch in range(nch):
                    dst = low_st[:, CH * ch:CH * (ch + 1)]
                    if ch % 2 == 0:
                        nc.scalar.copy(dst, acc[ch][:])
                    else:
                        nc.vector.tensor_copy(dst, acc[ch][:])
                # edge-column fixes (reflect at w=0 and w=191), both columns
                # per op via a stride-(wr-wl) length-2 inner dim
                out_ap = low_st[:, 0:stw].rearrange(
                    "p (r w) -> p r w", w=W)[:, :, 0:W:W - 1]
                for k in range(9):
                    di, dj = k // 3, k % 3
                    wl = (1, 0, 1)[dj]
                    wr = (190, 191, 190)[dj]
                    vb = PAD + (r0 + di) * W + wl
                    view = xt[:, vb:vb + rows * W].bitcast(f32).rearrange(
                        "p (r w) -> p r w", w=W)[:, :, 0:wr - wl + 1:wr - wl]
                    if k == 0:
                        nc.vector.tensor_scalar_mul(out_ap, view,
                                                    w128[:, 0:1])
                    else:
                        nc.vector.scalar_tensor_tensor(
                            out_ap, view, w128[:, k:k + 1], out_ap,
                            op0=mybir.AluOpType.mult,
                            op1=mybir.AluOpType.add)
                high_st = spool.tile([128, stw], f32, tag="high",
                                     padded_shape=[128, 3072])
                nc.vector.tensor_tensor(high_st[:],
                                        xt[:, base:base + stw].bitcast(f32),
                                        low_st[:],
                                        op=mybir.AluOpType.subtract)
                nc.scalar.dma_start(
                    dram_flat(low_d.ap().tensor, r0 * W, stw), low_st[:])
                nc.sync.dma_start(
                    dram_flat(high_d.ap().tensor, r0 * W, stw), high_st[:])
                r0 += rows

    nc.compile()
    return nc


def _enable_ldw_opt():
    """walrus emits one LDWEIGHTS per matmul with --enable-ldw-opt=false
    (72us of PE time for our 330 matmuls, mostly redundant reloads of the
    same diagonal).  Rewrite the flag on the compiler command line."""
    import concourse.bass_utils as BU
    if getattr(BU, "_ldw_patched", False):
        return
    orig = BU.run_command

    def patched(cmd, *a, **kw):
        cmd = [c.replace("--enable-ldw-opt=false", "--enable-ldw-opt=true")
               if isinstance(c, str) else c for c in cmd]
        return orig(cmd, *a, **kw)

    BU.run_command = patched
    BU._ldw_patched = True
    # bir_verify_and_optimise captured run_command at def time? (no - it
    # resolves the module global at call time, so the wrap is enough)


_nc_cache = None


def _get_program():
    global _nc_cache
    if _nc_cache is None:
        _enable_ldw_opt()
        _nc_cache = _build_program()
    return _nc_cache


def _host_consts(conv_w, bn_gamma, bn_beta, bn_mean, bn_var):
    s_a = bn_gamma / np.sqrt(bn_var + BN_EPS)
    b72 = (bn_beta - bn_mean * s_a).astype(np.float32).reshape(72, 1)
    A = (conv_w * s_a[:, None]) / np.float32(H * W)
    p = np.arange(128)
    at128 = np.ascontiguousarray(A.T[p // 2]).astype(np.float32)  # (128, 72)
    oc = np.arange(72)
    r9 = (oc[:, None] % 9 == np.arange(9)[None, :]).astype(np.float32)
    g728 = (oc[:, None] // 9 == np.arange(8)[None, :]).astype(np.float32)
    h8128 = (np.arange(8)[:, None] == (p[None, :] // 16)).astype(np.float32)
    eye = np.eye(128, dtype=np.float32)
    return dict(at128=at128, b72=b72, r9=r9, g728=g728, h8128=h8128, eye=eye)


def kernel(x, conv_w, bn_gamma, bn_beta, bn_mean, bn_var):
    x = np.ascontiguousarray(np.asarray(x, dtype=np.float32))
    consts = _host_consts(np.asarray(conv_w, np.float32),
                          np.asarray(bn_gamma, np.float32),
                          np.asarray(bn_beta, np.float32),
                          np.asarray(bn_mean, np.float32),
                          np.asarray(bn_var, np.float32))
    nc = _get_program()
    in_maps = [dict(x=x[i], **consts) for i in range(N)]
    res = run_bass_kernel_spmd(nc, in_maps, list(range(N))).results
    low = np.stack([res[i]["low"] for i in range(N)])
    high = np.stack([res[i]["high"] for i in range(N)])
    return low, high


if __name__ == "__main__":
    rng = np.random.default_rng(0)
    demo = dict(
        x=rng.standard_normal((N, IC, H, W), dtype=np.float32),
        conv_w=rng.standard_normal((72, 64)).astype(np.float32),
        bn_gamma=np.ones(72, np.float32),
        bn_beta=np.zeros(72, np.float32),
        bn_mean=rng.standard_normal(72).astype(np.float32) * 0.1,
        bn_var=rng.uniform(0.5, 1.5, 72).astype(np.float32),
    )
    low, high = kernel(**demo)
    print("ok", low.shape, high.shape)


# revision 17
# speedup vs baseline: 1.0831x; 1.0466x over previous
"""Trainium2 Bass kernel for dynamic low-pass filter decomposition.

Module: global-avg-pool -> 1x1 conv -> BN -> softmax over 3x3 taps gives a
per-(sample, group) 3x3 kernel; applied as a reflect-padded depthwise conv
over x; returns (low, x - low).

Sharding: data-parallel over batch n=8 across 8 NeuronCores (1 sample/core).

Per-core layout: partition p = h*64 + c (h = row-half of the image, c =
channel).  Each partition holds 98 rows x 192 cols of its (channel, half)
with one halo row above/below (reflection resolved at DMA time by source row
choice) plus a 1-element front/back pad so tap-shifted views stay in bounds.

The 9-tap weighted sum runs on the TensorEngine as 9 diagonal fp32r matmuls
per 512-column chunk accumulating in PSUM; ScalarE copies low out of PSUM,
VectorE computes high = x - low and fixes the reflect columns at w=0/191.
The softmax "kernel generation" runs on-device from exact per-ST partial
sums (fp32), with BN folded into the 1x1 conv weights on the host.
"""
import sys
import os

sys.path.insert(0, "/opt/trn_rl_repo")

import numpy as np
from contextlib import ExitStack

import concourse.bass as bass
import concourse.tile as tile
from concourse import bacc, mybir
from concourse.bass_utils import run_bass_kernel_spmd

dt = mybir.dt
f32 = dt.float32

KS = 3
GROUP = 8
IC = 64
BN_EPS = 1e-5
N = 8
H = W = 192
RH = 96                 # rows per half-image
NB = 98 * W             # buffer elems per partition (98 rows of 192)
PAD = 1                 # front pad elems (also 1 at the back)
NST = 6                 # input-phase tiles
STW = 3072              # cols per input tile
CH = 512                # cols per chunk (one PSUM bank)
ST_ROWS = [16, 16, 16, 16, 16, 8, 8]   # compute super-tile heights (rows)


def _build_program():
    """Trace the SPMD Bass program (same for every core)."""
    nc = bacc.Bacc("TRN2", target_bir_lowering=False, debug=False,
                   num_devices=N)

    x_d = nc.dram_tensor("x", [64, H, W], dt.float32r, kind="ExternalInput")
    at_d = nc.dram_tensor("at128", [128, 72], f32, kind="ExternalInput")
    b_d = nc.dram_tensor("b72", [72, 1], f32, kind="ExternalInput")
    r9_d = nc.dram_tensor("r9", [72, 9], f32, kind="ExternalInput")
    g_d = nc.dram_tensor("g728", [72, 8], f32, kind="ExternalInput")
    h_d = nc.dram_tensor("h8128", [8, 128], f32, kind="ExternalInput")
    eye_d = nc.dram_tensor("eye", [128, 128], f32, kind="ExternalInput")
    low_d = nc.dram_tensor("low", [64, H, W], f32, kind="ExternalOutput")
    high_d = nc.dram_tensor("high", [64, H, W], f32, kind="ExternalOutput")

    xt_dram = x_d.ap()

    def dram_flat(tensor, base, inner):
        """Flat (128, inner) AP over DRAM: partition p = c*2 + h covers
        x.flat[p*18432 + base : ... + inner].  Flat leading-dim-128 APs get
        the full 16-engine DMA spray (~305 GB/s); (h,c)-interleaved ones
        only engage 2 engines (~53 GB/s measured)."""
        return bass.AP(tensor, base, [[RH * W, 128], [1, inner]])

    with tile.TileContext(nc) as tc, ExitStack() as ctx:
        cpool = ctx.enter_context(tc.tile_pool(name="consts", bufs=1))
        xpool = ctx.enter_context(tc.tile_pool(name="x", bufs=1))
        wpool = ctx.enter_context(tc.tile_pool(name="w", bufs=1))
        spool = ctx.enter_context(tc.tile_pool(name="stage", bufs=3))

        # ---- x ST loads FIRST (queue FIFO position = landing time);
        # consts/halos after, so they don't delay the reduces ----
        xt = xpool.tile([128, PAD + NB + 1], dt.float32r)
        partials_v = wpool.tile([128, NST // 2], f32)
        partials_a = wpool.tile([128, NST // 2], f32)
        rscratch = wpool.tile([128, STW], f32)
        for s in range(NST):
            a = PAD + W + STW * s
            eng = nc.sync if s < 3 else nc.scalar
            eng.dma_start(xt[:, a:a + STW],
                          dram_flat(xt_dram.tensor, STW * s, STW))
        for s in range(NST):
            a = PAD + W + STW * s
            if s < 3:
                nc.vector.tensor_reduce(partials_v[:, s:s + 1],
                                        xt[:, a:a + STW].bitcast(f32),
                                        axis=mybir.AxisListType.X,
                                        op=mybir.AluOpType.add)
            else:
                nc.scalar.activation(rscratch[:],
                                     xt[:, a:a + STW].bitcast(f32),
                                     mybir.ActivationFunctionType.Copy,
                                     accum_out=partials_a[:, s - 3:s - 2])

        # ---- constant + halo loads (needed from the weight chain on) ----
        at_s = cpool.tile([128, 72], f32)
        b_s = cpool.tile([72, 1], f32)
        r9_s = cpool.tile([72, 9], f32)
        g_s = cpool.tile([72, 8], f32)
        h_s = cpool.tile([8, 128], f32)
        eye_s = cpool.tile([128, 128], f32)
        for t, d in ((at_s, at_d), (b_s, b_d), (r9_s, r9_d), (g_s, g_d),
                     (h_s, h_d), (eye_s, eye_d)):
            nc.scalar.dma_start(t[:], d.ap())
        # halo row 0 <- image rows {1 (reflect), 95}[h]
        nc.sync.dma_start(xt[:, PAD:PAD + W],
                          bass.AP(xt_dram.tensor, W,
                                  [[H * W, 64], [94 * W, 2], [1, W]]))
        # halo row 97 <- image rows {96, 190 (reflect)}[h]
        nc.sync.dma_start(xt[:, PAD + 97 * W:PAD + 98 * W],
                          bass.AP(xt_dram.tensor, 96 * W,
                                  [[H * W, 64], [94 * W, 2], [1, W]]))

        # ---- weight generation ----
        sum_v = wpool.tile([128, 1], f32)
        nc.vector.tensor_reduce(sum_v[:], partials_v[:],
                                axis=mybir.AxisListType.X,
                                op=mybir.AluOpType.add)
        sum_a = wpool.tile([128, 1], f32)
        nc.vector.tensor_reduce(sum_a[:], partials_a[:],
                                axis=mybir.AxisListType.X,
                                op=mybir.AluOpType.add)
        sum128 = wpool.tile([128, 1], f32)
        nc.vector.tensor_add(sum128[:], sum_v[:], sum_a[:])
        with tc.tile_pool(name="wpsum", bufs=1,
                          space=bass.MemorySpace.PSUM) as wpsum:
            lf_p = wpsum.tile([72, 1], f32, tag="lf")
            nc.tensor.matmul(lf_p[:], at_s[:], sum128[:])
            e72 = wpool.tile([72, 1], f32)
            nc.scalar.activation(e72[:], lf_p[:],
                                 mybir.ActivationFunctionType.Exp,
                                 bias=b_s[:, 0:1], scale=1.0)
            rhsw = wpool.tile([72, 9], f32)
            nc.vector.tensor_scalar_mul(rhsw[:], r9_s[:], e72[:, 0:1])
            w89_p = wpsum.tile([8, 9], f32, tag="w89")
            nc.tensor.matmul(w89_p[:], g_s[:], rhsw[:])
            s8 = wpool.tile([8, 1], f32)
            nc.vector.tensor_reduce(s8[:], w89_p[:],
                                    axis=mybir.AxisListType.X,
                                    op=mybir.AluOpType.add)
            r8 = wpool.tile([8, 1], f32)
            nc.vector.reciprocal(r8[:], s8[:])
            w89s = wpool.tile([8, 9], f32)
            nc.vector.tensor_scalar_mul(w89s[:], w89_p[:], r8[:, 0:1])
            wbig_p = wpsum.tile([128, 9], f32, tag="wbig")
            nc.tensor.matmul(wbig_p[:], h_s[:], w89s[:])
            w128 = wpool.tile([128, 9], f32)
            nc.scalar.copy(w128[:], wbig_p[:])

        # diagonal weight matrices, one tile per tap (separate tiles so
        # the first matmul only waits for its own diagonal); scalars read
        # straight from PSUM so PE needn't wait for the w128 SBUF copy
        diag = [wpool.tile([128, 128], dt.float32r, name=f"diag{k}")
                for k in range(9)]
        for k in range(9):
            nc.vector.tensor_scalar_mul(diag[k][:], eye_s[:],
                                        wbig_p[:, k:k + 1])

        # ---- main loop ----
        with tc.tile_pool(name="psum", bufs=8,
                          space=bass.MemorySpace.PSUM) as psum:
            r0 = 0
            for s, rows in enumerate(ST_ROWS):
                stw = rows * W
                nch = stw // CH
                base = PAD + W + r0 * W
                acc = [psum.tile([128, CH], f32, tag="acc", name=f"acc{s}_{i}")
                       for i in range(nch)]
                taps = range(9) if s % 2 == 0 else range(8, -1, -1)
                taps = list(taps)
                for k in taps:
                    di, dj = k // 3, k % 3
                    shift = (di - 1) * W + (dj - 1)
                    for ch in range(nch):
                        off = base + CH * ch + shift
                        nc.tensor.matmul(acc[ch][:], diag[k][:],
                                         xt[:, off:off + CH],
                                         start=(k == taps[0]),
                                         stop=(k == taps[-1]))
                low_st = spool.tile([128, stw], f32, tag="low",
                                    padded_shape=[128, 3072])
                for ch in range(nch):
                    dst = low_st[:, CH * ch:CH * (ch + 1)]
                    if ch % 2 == 0:
                        nc.scalar.copy(dst, acc[ch][:])
                    else:
                        nc.vector.tensor_copy(dst, acc[ch][:])
                # edge-column fixes (reflect at w=0 and w=191), both columns
                # per op via a stride-(wr-wl) length-2 inner dim
                out_ap = low_st[:, 0:stw].rearrange(
                    "p (r w) -> p r w", w=W)[:, :, 0:W:W - 1]
                for k in range(9):
                    di, dj = k // 3, k % 3
                    wl = (1, 0, 1)[dj]
                    wr = (190, 191, 190)[dj]
                    vb = PAD + (r0 + di) * W + wl
                    view = xt[:, vb:vb + rows * W].bitcast(f32).rearrange(
                        "p (r w) -> p r w", w=W)[:, :, 0:wr - wl + 1:wr - wl]
                    if k == 0:
                        nc.vector.tensor_scalar_mul(out_ap, view,
                                                    w128[:, 0:1])
                    else:
                        nc.vector.scalar_tensor_tensor(
                            out_ap, view, w128[:, k:k + 1], out_ap,
                            op0=mybir.AluOpType.mult,
                            op1=mybir.AluOpType.add)
                high_st = spool.tile([128, stw], f32, tag="high",
                                     padded_shape=[128, 3072])
                nc.vector.tensor_tensor(high_st[:],
                                        xt[:, base:base + stw].bitcast(f32),
                                        low_st[:],
                                        op=mybir.AluOpType.subtract)
                nc.scalar.dma_start(
                    dram_flat(low_d.ap().tensor, r0 * W, stw), low_st[:])
                nc.sync.dma_start(
                    dram_flat(high_d.ap().tensor, r0 * W, stw), high_st[:])
                r0 += rows

    nc.compile()
    return nc


def _enable_ldw_opt():
    """walrus emits one LDWEIGHTS per matmul with --enable-ldw-opt=false
    (72us of PE time for our 330 matmuls, mostly redundant reloads of the
    same diagonal).  Rewrite the flag on the compiler command line."""
    import concourse.bass_utils as BU
    if getattr(BU, "_ldw_patched", False):
        return
    orig = BU.run_command

    def patched(cmd, *a, **kw):
        cmd = [c.replace("--enable-ldw-opt=false", "--enable-ldw-opt=true")
               if isinstance(c, str) else c for c in cmd]
        return orig(cmd, *a, **kw)

    BU.run_command = patched
    BU._ldw_patched = True
    # bir_verify_and_optimise captured run_command at def time? (no - it
    # resolves the module global at call time, so the wrap is enough)


_nc_cache = None


def _get_program():
    global _nc_cache
    if _nc_cache is None:
        _enable_ldw_opt()
        _nc_cache = _build_program()
    return _nc_cache


def _host_consts(conv_w, bn_gamma, bn_beta, bn_mean, bn_var):
    s_a = bn_gamma / np.sqrt(bn_var + BN_EPS)
    b72 = (bn_beta - bn_mean * s_a).astype(np.float32).reshape(72, 1)
    A = (conv_w * s_a[:, None]) / np.float32(H * W)
    p = np.arange(128)
    at128 = np.ascontiguousarray(A.T[p // 2]).astype(np.float32)  # (128, 72)
    oc = np.arange(72)
    r9 = (oc[:, None] % 9 == np.arange(9)[None, :]).astype(np.float32)
    g728 = (oc[:, None] // 9 == np.arange(8)[None, :]).astype(np.float32)
    h8128 = (np.arange(8)[:, None] == (p[None, :] // 16)).astype(np.float32)
    eye = np.eye(128, dtype=np.float32)
    return dict(at128=at128, b72=b72, r9=r9, g728=g728, h8128=h8128, eye=eye)


def kernel(x, conv_w, bn_gamma, bn_beta, bn_mean, bn_var):
    x = np.ascontiguousarray(np.asarray(x, dtype=np.float32))
    consts = _host_consts(np.asarray(conv_w, np.float32),
                          np.asarray(bn_gamma, np.float32),
                          np.asarray(bn_beta, np.float32),
                          np.asarray(bn_mean, np.float32),
                          np.asarray(bn_var, np.float32))
    nc = _get_program()
    in_maps = [dict(x=x[i], **consts) for i in range(N)]
    res = run_bass_kernel_spmd(nc, in_maps, list(range(N))).results
    low = np.stack([res[i]["low"] for i in range(N)])
    high = np.stack([res[i]["high"] for i in range(N)])
    return low, high


if __name__ == "__main__":
    rng = np.random.default_rng(0)
    demo = dict(
        x=rng.standard_normal((N, IC, H, W), dtype=np.float32),
        conv_w=rng.standard_normal((72, 64)).astype(np.float32),
        bn_gamma=np.ones(72, np.float32),
        bn_beta=np.zeros(72, np.float32),
        bn_mean=rng.standard_normal(72).astype(np.float32) * 0.1,
        bn_var=rng.uniform(0.5, 1.5, 72).astype(np.float32),
    )
    low, high = kernel(**demo)
    print("ok", low.shape, high.shape)
